# revision 20
# baseline (speedup 1.0000x reference)
"""DotAttention (masked softmax attention) on 8 Trainium2 NeuronCores.

Strategy:
- Batch-parallel across cores: 32 batches -> 8 cores x 4 "slots".
  Batches are sorted by (padded) context length; slot k on every core
  processes one batch from rank group [8k, 8k+8), padded to the group max
  L_k (a compile-time constant shared by all cores -> one SPMD program).
- Per (slot, t-tile of 128 tgt rows):
    S = Q @ Ctx^T      3-pass bf16 hi/lo split matmul (fp32-grade scores)
                       + K=1 matmul adding -28672 to padded/masked columns
    softmax            exact fp32, flash-style: chunk-local max + fused
                       exp+sum at PSUM drain, rescale folded into normalize
    C = P @ Ctx        fp32r matmul; P^T via PE transpose
- Host precomputes transposed/split operands and masks; outputs are
  gathered/unsorted on host.
"""

import os
import numpy as np
import ml_dtypes

BATCH, TGT, SRC, DIM = 32, 1024, 4096, 1024
P = 128
KT = DIM // P            # 8 contraction k-tiles
CHUNK = 512              # src chunk (PSUM bank)
NEG = -28672.0           # bf16-exact large negative for masking
NCORES = 8
NSLOTS = 4
TT_PHASE = 4             # t-tiles per phase

_cache = {}


def _build(slot_lens, slot_mins):
    import concourse.bacc as bacc
    import concourse.mybir as mybir
    import concourse.tile as tile
    from concourse.masks import make_identity
    from contextlib import ExitStack

    dt = mybir.dt
    nc = bacc.Bacc("TRN2", target_bir_lowering=False)
    H = {}

    def dparam(name, shape, dtt, isOutput=False):
        H[name] = nc.declare_dram_parameter(name, shape, dtt, isOutput=isOutput)
        return H[name]

    for s, L in enumerate(slot_lens):
        dparam(f"qh{s}", [P, KT, TGT], dt.bfloat16)
        dparam(f"ql{s}", [P, KT, TGT], dt.bfloat16)
        dparam(f"cth{s}", [P, KT, L], dt.bfloat16)
        dparam(f"ctl{s}", [P, KT, L], dt.bfloat16)
        dparam(f"mask{s}", [1, L], dt.bfloat16)
        dparam(f"ctxn{s}", [L, DIM], dt.float16)
    dparam("align", [TGT, NSLOTS, SRC], dt.float32, isOutput=True)
    dparam("attn", [TGT, NSLOTS, DIM], dt.float32, isOutput=True)

    with tile.TileContext(nc) as tc, ExitStack() as ctx:
        pool = ctx.enter_context(tc.tile_pool(name="sbuf", bufs=2))
        qpool = ctx.enter_context(tc.tile_pool(name="qpool", bufs=1))
        spool = ctx.enter_context(tc.tile_pool(name="spool", bufs=1))
        cpool = ctx.enter_context(tc.tile_pool(name="cpool", bufs=3))
        ptpool = ctx.enter_context(tc.tile_pool(name="ptpool", bufs=3))
        psS = ctx.enter_context(tc.tile_pool(name="psS", bufs=2, space="PSUM"))
        psT = ctx.enter_context(tc.tile_pool(name="psT", bufs=2, space="PSUM"))
        psC = ctx.enter_context(tc.tile_pool(name="psC", bufs=1, space="PSUM"))
        consts = ctx.enter_context(tc.tile_pool(name="consts", bufs=1))

        ident = consts.tile([P, P], dt.float32)
        make_identity(nc, ident[:])
        ident_r = ident[:].bitcast(dt.float32r)
        ones1 = consts.tile([1, P], dt.bfloat16)
        nc.vector.memset(ones1[:], 1.0)
        zerot = consts.tile([P, 1024], dt.float32)
        nc.vector.memset(zerot[:], 0.0)

        for s, L in enumerate(slot_lens):
            NCH = L // CHUNK
            NS = L // P
            qh = qpool.tile([P, KT, TGT], dt.bfloat16, tag="qh")
            ql = qpool.tile([P, KT, TGT], dt.bfloat16, tag="ql")
            nc.sync.dma_start(qh[:], H[f"qh{s}"].ap())
            nc.sync.dma_start(ql[:], H[f"ql{s}"].ap())
            maskrow = qpool.tile([1, L], dt.bfloat16, tag="maskrow")
            nc.sync.dma_start(maskrow[:], H[f"mask{s}"].ap())

            for phase in range(TGT // (P * TT_PHASE)):
                Ssb = [spool.tile([P, L], dt.float32, tag=f"Ssb{i}", name=f"Ssb{i}")
                       for i in range(TT_PHASE)]
                cmax = [spool.tile([P, NCH], dt.float32, tag=f"cmax{i}", name=f"cmax{i}")
                        for i in range(TT_PHASE)]
                zpart = [spool.tile([P, NCH], dt.float32, tag=f"zpart{i}", name=f"zpart{i}")
                         for i in range(TT_PHASE)]
                # ---- QK: chunk-outer so ctxT streams once per phase ----
                for j in range(NCH):
                    cth = cpool.tile([P, KT, CHUNK], dt.bfloat16, tag="cth")
                    ctl = cpool.tile([P, KT, CHUNK], dt.bfloat16, tag="ctl")
                    nc.sync.dma_start(cth[:], H[f"cth{s}"].ap()[:, :, j * CHUNK:(j + 1) * CHUNK])
                    nc.sync.dma_start(ctl[:], H[f"ctl{s}"].ap()[:, :, j * CHUNK:(j + 1) * CHUNK])
                    for i in range(TT_PHASE):
                        t0 = (phase * TT_PHASE + i) * P
                        ps = psS.tile([P, CHUNK], dt.float32, tag="psS")
                        for k in range(KT):
                            nc.tensor.matmul(ps[:], qh[:, k, t0:t0 + P], cth[:, k],
                                             start=(k == 0), stop=False)
                        for k in range(KT):
                            nc.tensor.matmul(ps[:], qh[:, k, t0:t0 + P], ctl[:, k],
                                             start=False, stop=False)
                        need_mask = (j + 1) * CHUNK > slot_mins[s]
                        for k in range(KT):
                            nc.tensor.matmul(ps[:], ql[:, k, t0:t0 + P], cth[:, k],
                                             start=False,
                                             stop=(not need_mask and k == KT - 1))
                        if (j + 1) * CHUNK > slot_mins[s]:
                            nc.tensor.matmul(ps[:], ones1[:],
                                             maskrow[:, j * CHUNK:(j + 1) * CHUNK],
                                             start=False, stop=True)
                        # flash-style drain: chunk-local -max, then exp in the
                        # PSUM->SBUF move (rescaled to the global max later)
                        nc.vector.tensor_reduce(cmax[i][:, j:j + 1], ps[:],
                                                axis=mybir.AxisListType.X,
                                                op=mybir.AluOpType.max, negate=True)
                        nc.scalar.activation(Ssb[i][:, j * CHUNK:(j + 1) * CHUNK], ps[:],
                                             mybir.ActivationFunctionType.Exp,
                                             bias=cmax[i][:, j:j + 1], scale=1.0,
                                             accum_out=zpart[i][:, j:j + 1])

                # ---- softmax finalize: stats for all tiles first ----
                scs = []
                for i in range(TT_PHASE):
                    # cmax holds -m_j; negm = -max_j m_j = min_j(-m_j)
                    negm = pool.tile([P, 1], dt.float32, tag="negm")
                    nc.vector.tensor_reduce(negm[:], cmax[i][:], axis=mybir.AxisListType.X,
                                            op=mybir.AluOpType.min)
                    # esc_j = exp(m_j - m) = exp(-1 * (-m_j) + (-m))
                    esc = pool.tile([P, NCH], dt.float32, tag="esc")
                    nc.scalar.activation(esc[:], cmax[i][:],
                                         mybir.ActivationFunctionType.Exp,
                                         bias=negm[:], scale=-1.0)
                    ez = pool.tile([P, NCH], dt.float32, tag="ez")
                    nc.vector.tensor_mul(out=ez[:], in0=zpart[i][:], in1=esc[:])
                    Z = pool.tile([P, 1], dt.float32, tag="Z")
                    nc.vector.reduce_sum(Z[:], ez[:], axis=mybir.AxisListType.X)
                    rZ = pool.tile([P, 1], dt.float32, tag="rZ")
                    nc.vector.reciprocal(rZ[:], Z[:])
                    # per-chunk final scale = esc_j / Z
                    sc = pool.tile([P, NCH], dt.float32, tag=f"sc{i}", name=f"sc{i}")
                    nc.vector.tensor_scalar_mul(sc[:], esc[:], rZ[:])
                    scs.append(sc)
                # scale chunk-major so PV's first transposes unblock early
                for j in range(NCH):
                    for i in range(TT_PHASE):
                        nc.vector.tensor_scalar_mul(Ssb[i][:, j * CHUNK:(j + 1) * CHUNK],
                                                    Ssb[i][:, j * CHUNK:(j + 1) * CHUNK],
                                                    scs[i][:, j:j + 1])
                for i in range(TT_PHASE):
                    t0 = (phase * TT_PHASE + i) * P
                    nc.sync.dma_start(H["align"].ap()[t0:t0 + P, s, 0:L], Ssb[i][:])
                    z0 = L
                    while z0 < SRC:
                        zw = min(1024, SRC - z0)
                        nc.sync.dma_start(H["align"].ap()[t0:t0 + P, s, z0:z0 + zw],
                                          zerot[:, 0:zw])
                        z0 += zw

                # ---- PV: C[tt] = P[tt] @ ctxn ; d-halves, st-outer ----
                csb = [spool.tile([P, DIM], dt.float32, tag=f"csb{i}", name=f"csb{i}")
                       for i in range(TT_PHASE)]
                for sub in range(2):
                    pair = (2 * sub, 2 * sub + 1)
                    pc2 = [[psC.tile([P, CHUNK], dt.float32, tag=f"psC{u}{h}",
                                     name=f"psC{u}{h}") for h in range(2)]
                           for u in range(2)]

                    def stage(st, pair=pair):
                        ctxn = cpool.tile([P, DIM], dt.float16, tag="ctxn", name="ctxn")
                        nc.sync.dma_start(ctxn[:], H[f"ctxn{s}"].ap()[st * P:(st + 1) * P, :])
                        pt = psT.tile([P, 2 * P], dt.float32, tag="psT", name="pt")
                        for u in range(2):
                            nc.tensor.transpose(pt[:, u * P:(u + 1) * P],
                                                Ssb[pair[u]][:, st * P:(st + 1) * P], ident[:])
                        # per-tt half tiles on both engines: each matmul
                        # pair gates only on its own ~250ns copy
                        pta = ptpool.tile([P, P], dt.float16, tag="ptsbA", name="ptsbA")
                        ptb = ptpool.tile([P, P], dt.float16, tag="ptsbB", name="ptsbB")
                        nc.scalar.copy(pta[:], pt[:, 0:P])
                        nc.vector.tensor_copy(out=ptb[:], in_=pt[:, P:2 * P])
                        return ctxn, (pta, ptb)

                    def mms(st, ctxn, ptsb, pc2=pc2):
                        for u in range(2):
                            for h in range(2):
                                nc.tensor.matmul(pc2[u][h][:], ptsb[u][:],
                                                 ctxn[:, h * CHUNK:(h + 1) * CHUNK],
                                                 start=(st == 0), stop=(st == NS - 1))

                    # one-iteration software skew: st's transposes+copy are
                    # emitted (and prioritized) ahead of st-1's matmuls
                    prev = stage(0)
                    for st in range(1, NS):
                        cur = stage(st)
                        mms(st - 1, *prev)
                        prev = cur
                    mms(NS - 1, *prev)
                    for u in range(2):
                        for h in range(2):
                            nc.scalar.copy(csb[pair[u]][:, h * CHUNK:(h + 1) * CHUNK],
                                           pc2[u][h][:])
                for i in range(TT_PHASE):
                    t0 = (phase * TT_PHASE + i) * P
                    nc.sync.dma_start(H["attn"].ap()[t0:t0 + P, s, :], csb[i][:])

    nc.compile()
    return nc


def _split_bf16(x):
    h = x.astype(ml_dtypes.bfloat16)
    lo = (x - h.astype(np.float32)).astype(ml_dtypes.bfloat16)
    return h, lo


def _prep_core_inputs(inp, ctx, lengths, batches, slot_lens):
    """Build one core's input map. batches[k] = batch index for slot k."""
    m = {}
    for s, (b, L) in enumerate(zip(batches, slot_lens)):
        q = inp[b]                      # [TGT, DIM]
        qT = np.ascontiguousarray(q.T)  # [DIM, TGT]
        qh, ql = _split_bf16(qT.reshape(KT, P, TGT).transpose(1, 0, 2))
        m[f"qh{s}"] = np.ascontiguousarray(qh)
        m[f"ql{s}"] = np.ascontiguousarray(ql)
        c = ctx[b][:L]                  # [L, DIM]
        cT = np.ascontiguousarray(c.T)  # [DIM, L]
        ch, cl = _split_bf16(cT.reshape(KT, P, L).transpose(1, 0, 2))
        m[f"cth{s}"] = np.ascontiguousarray(ch)
        m[f"ctl{s}"] = np.ascontiguousarray(cl)
        mask = np.zeros((1, L), dtype=ml_dtypes.bfloat16)
        l = int(lengths[b])
        if l < L:
            mask[0, l:] = NEG
        m[f"mask{s}"] = mask
        m[f"ctxn{s}"] = np.ascontiguousarray(c.astype(np.float16))
    return m


def _plan(lengths):
    padded = [max(CHUNK, -(-int(l) // CHUNK) * CHUNK) for l in lengths]
    order = sorted(range(BATCH), key=lambda b: -padded[b])
    groups = [order[8 * k:8 * k + 8] for k in range(NSLOTS)]
    slot_lens = tuple(padded[g[0]] for g in groups)
    slot_mins = tuple(min(int(lengths[b]) for b in g) for g in groups)
    # core c takes groups[k][c] for slot k
    core_batches = [[groups[k][c] for k in range(NSLOTS)] for c in range(NCORES)]
    return slot_lens, slot_mins, core_batches


def kernel(input, context, context_lengths, _trace=False):
    from concourse.bass_utils import run_bass_kernel_spmd

    inp = np.ascontiguousarray(input, dtype=np.float32)
    ctx = np.ascontiguousarray(context, dtype=np.float32)
    lengths = np.asarray(context_lengths)

    slot_lens, slot_mins, core_batches = _plan(lengths)
    key = (slot_lens, slot_mins)
    if key not in _cache:
        _cache[key] = _build(slot_lens, slot_mins)
    nc = _cache[key]

    in_maps = [_prep_core_inputs(inp, ctx, lengths, core_batches[c], slot_lens)
               for c in range(NCORES)]
    kw = {}
    if _trace:
        kw = dict(trace=True, stitch_traces=False)
    out = run_bass_kernel_spmd(nc, in_maps, core_ids=list(range(NCORES)), **kw)

    attn = np.empty((TGT, BATCH, DIM), dtype=np.float32)
    align = np.empty((TGT, BATCH, SRC), dtype=np.float32)
    for c in range(NCORES):
        r = out.results[c]
        for k in range(NSLOTS):
            b = core_batches[c][k]
            attn[:, b, :] = r["attn"][:, k, :]
            align[:, b, :] = r["align"][:, k, :]
    if _trace:
        kernel.last_exec_ns = out.exec_time_ns
        kernel.last_results = out
    return attn, align


# revision 22
# speedup vs baseline: 1.0755x; 1.0755x over previous
"""DotAttention (masked softmax attention) on 8 Trainium2 NeuronCores.

Strategy:
- Batch-parallel across cores: 32 batches -> 8 cores x 4 "slots".
  Batches are sorted by (padded) context length; slot k on every core
  processes one batch from rank group [8k, 8k+8), padded to the group max
  L_k (a compile-time constant shared by all cores -> one SPMD program).
- Per (slot, t-tile of 128 tgt rows):
    S = Q @ Ctx^T      3-pass bf16 hi/lo split matmul (fp32-grade scores)
                       + K=1 matmul adding -28672 to padded/masked columns
    softmax            exact fp32, flash-style: chunk-local max + fused
                       exp+sum at PSUM drain, rescale folded into normalize
    C = P @ Ctx        fp32r matmul; P^T via PE transpose
- Host precomputes transposed/split operands and masks; outputs are
  gathered/unsorted on host.
"""

import os
import numpy as np
import ml_dtypes

BATCH, TGT, SRC, DIM = 32, 1024, 4096, 1024
P = 128
KT = DIM // P            # 8 contraction k-tiles
CHUNK = 512              # src chunk (PSUM bank)
NEG = -28672.0           # bf16-exact large negative for masking
NCORES = 8
NSLOTS = 4
TT_PHASE = 4             # t-tiles per phase

_cache = {}


def _build(slot_lens, slot_mins):
    import concourse.bacc as bacc
    import concourse.mybir as mybir
    import concourse.tile as tile
    from concourse.masks import make_identity
    from contextlib import ExitStack

    dt = mybir.dt
    nc = bacc.Bacc("TRN2", target_bir_lowering=False)
    H = {}

    def dparam(name, shape, dtt, isOutput=False):
        H[name] = nc.declare_dram_parameter(name, shape, dtt, isOutput=isOutput)
        return H[name]

    for s, L in enumerate(slot_lens):
        dparam(f"qh{s}", [P, KT, TGT], dt.bfloat16)
        dparam(f"ql{s}", [P, KT, TGT], dt.bfloat16)
        dparam(f"cth{s}", [P, KT, L], dt.bfloat16)
        dparam(f"ctl{s}", [P, KT, L], dt.bfloat16)
        dparam(f"mask{s}", [1, L], dt.bfloat16)
        dparam(f"ctxn{s}", [L, DIM], dt.float16)
    dparam("align", [TGT, NSLOTS, SRC], dt.float32, isOutput=True)
    dparam("attn", [TGT, NSLOTS, DIM], dt.float32, isOutput=True)

    with tile.TileContext(nc) as tc, ExitStack() as ctx:
        pool = ctx.enter_context(tc.tile_pool(name="sbuf", bufs=2))
        qpool = ctx.enter_context(tc.tile_pool(name="qpool", bufs=1))
        spool = ctx.enter_context(tc.tile_pool(name="spool", bufs=1))
        cpool = ctx.enter_context(tc.tile_pool(name="cpool", bufs=3))
        ptpool = ctx.enter_context(tc.tile_pool(name="ptpool", bufs=3))
        ctxpool = ctx.enter_context(tc.tile_pool(name="ctxpool", bufs=6))
        psS = ctx.enter_context(tc.tile_pool(name="psS", bufs=2, space="PSUM"))
        psT = ctx.enter_context(tc.tile_pool(name="psT", bufs=2, space="PSUM"))
        psC = ctx.enter_context(tc.tile_pool(name="psC", bufs=1, space="PSUM"))
        consts = ctx.enter_context(tc.tile_pool(name="consts", bufs=1))

        ident = consts.tile([P, P], dt.float32)
        make_identity(nc, ident[:])
        ident_r = ident[:].bitcast(dt.float32r)
        ones1 = consts.tile([1, P], dt.bfloat16)
        nc.vector.memset(ones1[:], 1.0)
        zerot = consts.tile([P, 1024], dt.float32)
        nc.vector.memset(zerot[:], 0.0)

        for s, L in enumerate(slot_lens):
            NCH = L // CHUNK
            NS = L // P
            qh = qpool.tile([P, KT, TGT], dt.bfloat16, tag="qh")
            ql = qpool.tile([P, KT, TGT], dt.bfloat16, tag="ql")
            HT = TGT // 2
            nc.sync.dma_start(qh[:, :, 0:HT], H[f"qh{s}"].ap()[:, :, 0:HT])
            nc.sync.dma_start(ql[:, :, 0:HT], H[f"ql{s}"].ap()[:, :, 0:HT])
            nc.sync.dma_start(qh[:, :, HT:TGT], H[f"qh{s}"].ap()[:, :, HT:TGT])
            nc.sync.dma_start(ql[:, :, HT:TGT], H[f"ql{s}"].ap()[:, :, HT:TGT])
            maskrow = qpool.tile([1, L], dt.bfloat16, tag="maskrow")
            nc.sync.dma_start(maskrow[:], H[f"mask{s}"].ap())

            for phase in range(TGT // (P * TT_PHASE)):
                Ssb = [spool.tile([P, L], dt.float32, tag=f"Ssb{i}", name=f"Ssb{i}")
                       for i in range(TT_PHASE)]
                cmax = [spool.tile([P, NCH], dt.float32, tag=f"cmax{i}", name=f"cmax{i}")
                        for i in range(TT_PHASE)]
                zpart = [spool.tile([P, NCH], dt.float32, tag=f"zpart{i}", name=f"zpart{i}")
                         for i in range(TT_PHASE)]
                # ---- QK: chunk-outer so ctxT streams once per phase ----
                for j in range(NCH):
                    cth = cpool.tile([P, KT, CHUNK], dt.bfloat16, tag="cth")
                    ctl = cpool.tile([P, KT, CHUNK], dt.bfloat16, tag="ctl")
                    nc.sync.dma_start(cth[:], H[f"cth{s}"].ap()[:, :, j * CHUNK:(j + 1) * CHUNK])
                    nc.sync.dma_start(ctl[:], H[f"ctl{s}"].ap()[:, :, j * CHUNK:(j + 1) * CHUNK])
                    for i in range(TT_PHASE):
                        t0 = (phase * TT_PHASE + i) * P
                        ps = psS.tile([P, CHUNK], dt.float32, tag="psS")
                        for k in range(KT):
                            nc.tensor.matmul(ps[:], qh[:, k, t0:t0 + P], cth[:, k],
                                             start=(k == 0), stop=False)
                        for k in range(KT):
                            nc.tensor.matmul(ps[:], qh[:, k, t0:t0 + P], ctl[:, k],
                                             start=False, stop=False)
                        need_mask = (j + 1) * CHUNK > slot_mins[s]
                        for k in range(KT):
                            nc.tensor.matmul(ps[:], ql[:, k, t0:t0 + P], cth[:, k],
                                             start=False,
                                             stop=(not need_mask and k == KT - 1))
                        if (j + 1) * CHUNK > slot_mins[s]:
                            nc.tensor.matmul(ps[:], ones1[:],
                                             maskrow[:, j * CHUNK:(j + 1) * CHUNK],
                                             start=False, stop=True)
                        # flash-style drain: chunk-local -max, then exp in the
                        # PSUM->SBUF move (rescaled to the global max later)
                        nc.vector.tensor_reduce(cmax[i][:, j:j + 1], ps[:],
                                                axis=mybir.AxisListType.X,
                                                op=mybir.AluOpType.max, negate=True)
                        nc.scalar.activation(Ssb[i][:, j * CHUNK:(j + 1) * CHUNK], ps[:],
                                             mybir.ActivationFunctionType.Exp,
                                             bias=cmax[i][:, j:j + 1], scale=1.0,
                                             accum_out=zpart[i][:, j:j + 1])

                # ---- softmax finalize: stats for all tiles first ----
                scs = []
                for i in range(TT_PHASE):
                    # cmax holds -m_j; negm = -max_j m_j = min_j(-m_j)
                    negm = pool.tile([P, 1], dt.float32, tag="negm")
                    nc.vector.tensor_reduce(negm[:], cmax[i][:], axis=mybir.AxisListType.X,
                                            op=mybir.AluOpType.min)
                    # esc_j = exp(m_j - m) = exp(-1 * (-m_j) + (-m))
                    esc = pool.tile([P, NCH], dt.float32, tag="esc")
                    nc.scalar.activation(esc[:], cmax[i][:],
                                         mybir.ActivationFunctionType.Exp,
                                         bias=negm[:], scale=-1.0)
                    ez = pool.tile([P, NCH], dt.float32, tag="ez")
                    nc.vector.tensor_mul(out=ez[:], in0=zpart[i][:], in1=esc[:])
                    Z = pool.tile([P, 1], dt.float32, tag="Z")
                    nc.vector.reduce_sum(Z[:], ez[:], axis=mybir.AxisListType.X)
                    rZ = pool.tile([P, 1], dt.float32, tag="rZ")
                    nc.vector.reciprocal(rZ[:], Z[:])
                    # per-chunk final scale = esc_j / Z
                    sc = pool.tile([P, NCH], dt.float32, tag=f"sc{i}", name=f"sc{i}")
                    nc.vector.tensor_scalar_mul(sc[:], esc[:], rZ[:])
                    scs.append(sc)
                # scale chunk-major so PV's first transposes unblock early
                for j in range(NCH):
                    for i in range(TT_PHASE):
                        nc.vector.tensor_scalar_mul(Ssb[i][:, j * CHUNK:(j + 1) * CHUNK],
                                                    Ssb[i][:, j * CHUNK:(j + 1) * CHUNK],
                                                    scs[i][:, j:j + 1])
                for i in range(TT_PHASE):
                    t0 = (phase * TT_PHASE + i) * P
                    nc.sync.dma_start(H["align"].ap()[t0:t0 + P, s, 0:L], Ssb[i][:])
                    z0 = L
                    while z0 < SRC:
                        zw = min(1024, SRC - z0)
                        nc.sync.dma_start(H["align"].ap()[t0:t0 + P, s, z0:z0 + zw],
                                          zerot[:, 0:zw])
                        z0 += zw

                # ---- PV: C[tt] = P[tt] @ ctxn ; d-halves, st-outer ----
                csb = [spool.tile([P, DIM], dt.float32, tag=f"csb{i}", name=f"csb{i}")
                       for i in range(TT_PHASE)]
                for sub in range(2):
                    pair = (2 * sub, 2 * sub + 1)
                    pc2 = [[psC.tile([P, CHUNK], dt.float32, tag=f"psC{u}{h}",
                                     name=f"psC{u}{h}") for h in range(2)]
                           for u in range(2)]

                    def stage(st, pair=pair):
                        ctxn = ctxpool.tile([P, DIM], dt.float16, tag="ctxn", name="ctxn")
                        nc.sync.dma_start(ctxn[:], H[f"ctxn{s}"].ap()[st * P:(st + 1) * P, :])
                        pt = psT.tile([P, 2 * P], dt.float32, tag="psT", name="pt")
                        for u in range(2):
                            nc.tensor.transpose(pt[:, u * P:(u + 1) * P],
                                                Ssb[pair[u]][:, st * P:(st + 1) * P], ident[:])
                        ptsb = ptpool.tile([P, 2 * P], dt.float16, tag="ptsb", name="ptsb")
                        if st % 2 == 0:
                            nc.vector.tensor_copy(out=ptsb[:], in_=pt[:])
                        else:
                            nc.scalar.copy(ptsb[:], pt[:])
                        return ctxn, ptsb

                    def mms(st, ctxn, ptsb, pc2=pc2):
                        for u in range(2):
                            for h in range(2):
                                nc.tensor.matmul(pc2[u][h][:], ptsb[:, u * P:(u + 1) * P],
                                                 ctxn[:, h * CHUNK:(h + 1) * CHUNK],
                                                 start=(st == 0), stop=(st == NS - 1))

                    # one-iteration software skew: st's transposes+copy are
                    # emitted (and prioritized) ahead of st-1's matmuls
                    prev = stage(0)
                    for st in range(1, NS):
                        cur = stage(st)
                        mms(st - 1, *prev)
                        prev = cur
                    mms(NS - 1, *prev)
                    for u in range(2):
                        for h in range(2):
                            nc.scalar.copy(csb[pair[u]][:, h * CHUNK:(h + 1) * CHUNK],
                                           pc2[u][h][:])
                for i in range(TT_PHASE):
                    t0 = (phase * TT_PHASE + i) * P
                    nc.sync.dma_start(H["attn"].ap()[t0:t0 + P, s, :], csb[i][:])

    nc.compile()
    return nc


def _split_bf16(x):
    h = x.astype(ml_dtypes.bfloat16)
    lo = (x - h.astype(np.float32)).astype(ml_dtypes.bfloat16)
    return h, lo


def _prep_core_inputs(inp, ctx, lengths, batches, slot_lens):
    """Build one core's input map. batches[k] = batch index for slot k."""
    m = {}
    for s, (b, L) in enumerate(zip(batches, slot_lens)):
        q = inp[b]                      # [TGT, DIM]
        qT = np.ascontiguousarray(q.T)  # [DIM, TGT]
        qh, ql = _split_bf16(qT.reshape(KT, P, TGT).transpose(1, 0, 2))
        m[f"qh{s}"] = np.ascontiguousarray(qh)
        m[f"ql{s}"] = np.ascontiguousarray(ql)
        c = ctx[b][:L]                  # [L, DIM]
        cT = np.ascontiguousarray(c.T)  # [DIM, L]
        ch, cl = _split_bf16(cT.reshape(KT, P, L).transpose(1, 0, 2))
        m[f"cth{s}"] = np.ascontiguousarray(ch)
        m[f"ctl{s}"] = np.ascontiguousarray(cl)
        mask = np.zeros((1, L), dtype=ml_dtypes.bfloat16)
        l = int(lengths[b])
        if l < L:
            mask[0, l:] = NEG
        m[f"mask{s}"] = mask
        m[f"ctxn{s}"] = np.ascontiguousarray(c.astype(np.float16))
    return m


def _plan(lengths):
    padded = [max(CHUNK, -(-int(l) // CHUNK) * CHUNK) for l in lengths]
    order = sorted(range(BATCH), key=lambda b: -padded[b])
    groups = [order[8 * k:8 * k + 8] for k in range(NSLOTS)]
    slot_lens = tuple(padded[g[0]] for g in groups)
    slot_mins = tuple(min(int(lengths[b]) for b in g) for g in groups)
    # core c takes groups[k][c] for slot k
    core_batches = [[groups[k][c] for k in range(NSLOTS)] for c in range(NCORES)]
    return slot_lens, slot_mins, core_batches


def kernel(input, context, context_lengths, _trace=False):
    from concourse.bass_utils import run_bass_kernel_spmd

    inp = np.ascontiguousarray(input, dtype=np.float32)
    ctx = np.ascontiguousarray(context, dtype=np.float32)
    lengths = np.asarray(context_lengths)

    slot_lens, slot_mins, core_batches = _plan(lengths)
    key = (slot_lens, slot_mins)
    if key not in _cache:
        _cache[key] = _build(slot_lens, slot_mins)
    nc = _cache[key]

    in_maps = [_prep_core_inputs(inp, ctx, lengths, core_batches[c], slot_lens)
               for c in range(NCORES)]
    kw = {}
    if _trace:
        kw = dict(trace=True, stitch_traces=False)
    out = run_bass_kernel_spmd(nc, in_maps, core_ids=list(range(NCORES)), **kw)

    attn = np.empty((TGT, BATCH, DIM), dtype=np.float32)
    align = np.empty((TGT, BATCH, SRC), dtype=np.float32)
    for c in range(NCORES):
        r = out.results[c]
        for k in range(NSLOTS):
            b = core_batches[c][k]
            attn[:, b, :] = r["attn"][:, k, :]
            align[:, b, :] = r["align"][:, k, :]
    if _trace:
        kernel.last_exec_ns = out.exec_time_ns
        kernel.last_results = out
    return attn, align
